# revision 34
# baseline (speedup 1.0000x reference)
"""DCN (cross+deep) Trainium2 Bass kernel, 8 NeuronCores.

Sharding: data-parallel over batch (2048 rows/core); embedding gather on
host (table never touches the device); cross/deep weights replicated.

Key structure (vs the naive formulation):
  * Cross branch is algebraically collapsed: with a_i = x0 . w_i and
    a_3 = x0 . ow_cross, the full cross stack + its output contribution
    reduce to per-row scalar recurrences:
       S0 = a0; u1 = 1+S0; S1 = u1*a1 + c1; u2 = u1+S1; S2 = u2*a2 + c2;
       T = u2+S2; out_cross = T*a3 + const.
    So the PE computes ONE 7-matmul group ([128,4] lhsT) instead of
    3x7 broadcast matvecs + 7 output matvecs.
  * Deep branch runs in fp8(e4m3) with DoubleRow perf mode: each matmul
    contracts two 128-row k-tiles at 0.5 cycles/output-row. Activations
    are scaled x256 and weights x16 (exact power-of-2 descale in the
    relus), keeping everything in e4m3's normal range.
  * x ships pre-transposed from host in bf16 (cross) + fp8 (deep)
    layouts. Engine split per chunk: ACT = L0 relus + a-copy; DVE =
    L1/L2 relus + final add; Pool = cross recurrence ([4,128] layout,
    brought to partitions 0-3 by a tiny SBUF->SBUF DMA shuffle).
  * L1/L2 run k-pair-outer so they can start as soon as the first two
    producer tiles are relu'd; out_d accumulates in [4,128] PSUM groups
    so the tail is one small DVE add + DMA.
"""

import numpy as np
import ml_dtypes
from contextlib import ExitStack

import concourse.tile as tile
import concourse.mybir as mybir
from concourse import bacc
from concourse.bass_utils import run_bass_kernel_spmd

# ---- problem constants (hardcoded; kernel.py must be self-contained) ----
B, F, E = 16384, 26, 32
NF = 1_000_000
D = F * E                     # 832
DEEP = (1024, 512, 256)
N_CORES = 8
S = B // N_CORES              # 2048 rows per core
CHUNK = 512
NCHUNK = S // CHUNK           # 4
KB = 7                        # bf16 k-tiles (896 = 28 features)
K8 = 8                        # fp8 k-tiles (1024 = 32 features)
FPB, FP8 = 28, 32             # padded feature counts
M0, M1, M2 = DEEP[0] // 128, DEEP[1] // 128, DEEP[2] // 128  # 8, 4, 2
XS, WS = 256.0, 16.0          # fp8 scales for activations / weights

_bf = mybir.dt.bfloat16
_f8 = mybir.dt.float8e4
_f32 = mybir.dt.float32
_np_bf = ml_dtypes.bfloat16
_np_f8 = ml_dtypes.float8_e4m3

_CACHE = {}
DR = mybir.MatmulPerfMode.DoubleRow


def _build_nc(zb=True, zc=True, zo=True):
    """zb: deep biases all zero; zc: cross biases zero; zo: out bias zero."""
    AF = mybir.ActivationFunctionType
    OP = mybir.AluOpType
    nc = bacc.Bacc(
        "TRN2", target_bir_lowering=False, debug=False, num_devices=N_CORES
    )

    # x pre-transposed on host: xtb[p, k*S+b] = bf16(x[b, k*128+p])
    xtb_d = nc.dram_tensor("xtb", [128, KB * S], _bf, kind="ExternalInput")
    # x8[p, k*S+b] = fp8(x[b, k*128+p] * 256)
    xt8_d = nc.dram_tensor("xt8", [128, K8 * S], _f8, kind="ExternalInput")
    # deep weights fp8 (x16): w[p, k, m] = fp8(W[k*128+p, m] * 16)
    w08_d = nc.dram_tensor("w08", [128, K8 * DEEP[0]], _f8, kind="ExternalInput")
    w18_d = nc.dram_tensor("w18", [128, K8 * DEEP[1]], _f8, kind="ExternalInput")
    w28_d = nc.dram_tensor("w28", [128, M1 * DEEP[2]], _f8, kind="ExternalInput")
    # fp8 deep-out column pair (x16); psO comes out scaled by 4096 and the
    # host divides the final output by 4096 (exact power of two)
    ow8_d = nc.dram_tensor("ow8", [128, M2], _f8, kind="ExternalInput")
    # merged small weights bf16: [cwo (28) | owd (2) | obb (1) | vcol (4)]
    # vcol: partition-0 row [1,1,1,0] -- adds +1 to a0..a2 inside the a-pass
    # psum group (via a ones-rhs matmul), so psA rows become [v0,v1,v2,a3]
    # with v_i = 1 + a_i and the cross recurrence factorizes to
    #   out_cross = ((v0*v1 + c1)*v2 + c2) * a3.
    SMW = KB * 4 + M2 + 1 + 4
    smw_d = nc.dram_tensor("smw", [128, SMW], _bf, kind="ExternalInput")
    if not zc:
        sc_d = nc.dram_tensor("sc", [1, 2], _f32, kind="ExternalInput")
    if not zb:
        cst_d = nc.dram_tensor("cst", [128, M0 + M1 + M2], _f32, kind="ExternalInput")
    out_d = nc.dram_tensor("out", [NCHUNK, CHUNK], _f32, kind="ExternalOutput")

    xtb_r = xtb_d[:, :].rearrange("p (k s) -> p k s", k=KB)
    xt8_r = xt8_d[:, :].rearrange("p (k s) -> p k s", k=K8)
    w08_r = w08_d[:, :].rearrange("p (k m) -> p k m", k=K8)
    w18_r = w18_d[:, :].rearrange("p (k m) -> p k m", k=K8)
    w28_r = w28_d[:, :].rearrange("p (k m) -> p k m", k=M1)

    with ExitStack() as ctx:
        tc = ctx.enter_context(tile.TileContext(nc))
        wp = ctx.enter_context(tc.tile_pool(name="wp", bufs=1))
        xbp = ctx.enter_context(tc.tile_pool(name="xbp", bufs=2))
        x8p = ctx.enter_context(tc.tile_pool(name="x8p", bufs=2))
        yp = ctx.enter_context(tc.tile_pool(name="yp", bufs=2))
        asp = ctx.enter_context(tc.tile_pool(name="asp", bufs=2))
        rp = ctx.enter_context(tc.tile_pool(name="rp", bufs=2))
        otp = ctx.enter_context(tc.tile_pool(name="otp", bufs=2))
        dps = ctx.enter_context(tc.tile_pool(name="dps", bufs=4, space="PSUM"))
        aps = ctx.enter_context(tc.tile_pool(name="aps", bufs=1, space="PSUM"))
        ops = ctx.enter_context(tc.tile_pool(name="ops", bufs=2, space="PSUM"))

        # ---- x chunk 0 + first half of w0 first: L0 m0-3 critical path.
        # w0 lives in TWO tiles so L0 m0-3 don't wait on the second DMA
        # (tile dependency tracking is tile-granular). ----
        xt8_0 = x8p.tile([128, K8, CHUNK], _f8, tag="xt8", name="xt8_0")
        nc.sync.dma_start(xt8_0[:], xt8_r[:, :, 0:CHUNK])
        w08a_sb = wp.tile([128, K8, DEEP[0] // 2], _f8)
        w08b_sb = wp.tile([128, K8, DEEP[0] // 2], _f8)
        w18_sb = wp.tile([128, K8, DEEP[1]], _f8)
        w28_sb = wp.tile([128, M1, DEEP[2]], _f8)
        ow8_sb = wp.tile([128, M2, 1], _f8)
        nc.sync.dma_start(w08a_sb[:], w08_r[:, :, 0:512])
        nc.sync.dma_start(w08b_sb[:], w08_r[:, :, 512:1024])

        def w0l(m):  # [128, 2, 128] lhsT slice provider for L0 tile (j pair)
            t = w08a_sb if m < 4 else w08b_sb
            mm = m % 4
            return lambda j: t[:, 2 * j:2 * j + 2, mm * 128:(mm + 1) * 128]

        smw_sb = wp.tile([128, SMW], _bf)
        nc.sync.dma_start(smw_sb[:], smw_d[:, :])

        def cwo(k):  # [128, 4] lhsT for a-pass k-tile
            return smw_sb[:, k * 4:(k + 1) * 4]

        def owd(m):  # [128, 1] deep-out column
            return smw_sb[:, KB * 4 + m:KB * 4 + m + 1]

        obb = smw_sb[:, KB * 4 + M2:KB * 4 + M2 + 1]
        vcol = smw_sb[:, KB * 4 + M2 + 1:KB * 4 + M2 + 5]
        if not zc:
            sc_sb = wp.tile([1, 2], _f32)
            nc.sync.dma_start(sc_sb[:], sc_d[:, :])
        if not zb:
            cst_sb = wp.tile([128, M0 + M1 + M2], _f32)
            nc.sync.dma_start(cst_sb[:], cst_d[:, :])
            b0_sb = cst_sb[:, 0:M0]
            b1_sb = cst_sb[:, M0:M0 + M1]
            b2_sb = cst_sb[:, M0 + M1:M0 + M1 + M2]

        # ---- preamble: observe ops + PE warm-up (p-state ramp) ----
        obs = wp.tile([128, 8], _f32)
        nc.vector.tensor_copy(obs[:, 0:1], smw_sb[:, 0:1])
        nc.gpsimd.tensor_copy(obs[:, 1:2], smw_sb[:, 0:1])
        if not zc:
            nc.vector.tensor_copy(obs[:, 2:3], sc_sb[0:1, 0:1])
        nc.scalar.activation(obs[:, 3:4], smw_sb[:, 0:1], AF.Copy)
        if not zb:
            nc.scalar.activation(obs[:, 4:5], b0_sb[:, 0:1], AF.Copy)
        warm = wp.tile([128, 512], _bf)
        nc.vector.memset(warm[:], 0.0)
        ones_sb = wp.tile([128, CHUNK], _bf)
        nc.gpsimd.memset(ones_sb[:], 1.0)
        warm_ps = dps.tile([128, 512], _f32, tag="dps", name="warm_ps")
        for _ in range(8):
            nc.tensor.matmul(
                warm_ps[:], lhsT=warm[:, 0:128], rhs=warm[:], start=True, stop=True
            )
        # NOTE: only touch tensors whose DMAs are emitted BEFORE this point —
        # touching late-loaded weights stalls the in-order PE stream on their
        # DMA semaphores.
        dummy_ps = ops.tile([1, 8], _f32, tag="dummy", bufs=1)
        touch = [
            w08a_sb[:, 0, 0:1],
            smw_sb[:, 0:1],
            ones_sb[:, 0:1],
        ]
        for w_ap in touch:
            nc.tensor.matmul(dummy_ps[0:1, 0:1], lhsT=w_ap, rhs=w_ap, start=True, stop=True)

        HH = CHUNK // 2

        def relu(out_ap, ps, scale, bias_col):
            # fp8/bf16 relu of a [128, CHUNK] psum tile, split column-wise
            # across ACT and DVE so output latency ~ half an op.
            if zb:
                nc.scalar.activation(
                    out_ap[:, 0:HH], ps[:, 0:HH], AF.Relu, scale=scale
                )
                nc.vector.tensor_scalar(
                    out_ap[:, HH:], ps[:, HH:], scale, 0.0, OP.mult, OP.max
                )
            else:
                nc.scalar.activation(
                    out_ap[:, :], ps[:, :], AF.Relu, bias=bias_col, scale=scale
                )

        for c in range(NCHUNK):
            cs = c * CHUNK
            if c == 0:
                xt8_t = xt8_0
            else:
                xt8_t = x8p.tile([128, K8, CHUNK], _f8, tag="xt8", name=f"xt8_{c}")
                nc.scalar.dma_start(xt8_t[:], xt8_r[:, :, cs:cs + CHUNK])
            xtb_t = xbp.tile([128, KB, CHUNK], _bf, tag="xtb", name=f"xtb_{c}")
            (nc.sync if c == 0 else nc.scalar).dma_start(
                xtb_t[:], xtb_r[:, :, cs:cs + CHUNK]
            )

            # ---- deep L0 (fp8 DoubleRow), psum = h0 * 4096 ----
            y0t = yp.tile([128, K8, CHUNK], _f8, tag="y0", name=f"y0_{c}")
            for m in range(M0):
                ps = dps.tile([128, CHUNK], _f32, tag="dps", name=f"ps0_{c}_{m}")
                lhs = w0l(m)
                for j in range(K8 // 2):
                    nc.tensor.matmul(
                        ps[:],
                        lhsT=lhs(j),
                        rhs=xt8_t[:, 2 * j:2 * j + 2, :],
                        start=(j == 0),
                        stop=(j == K8 // 2 - 1),
                        perf_mode=DR,
                    )
                # y0 = fp8(relu(h0)*256) = relu(psum/16 [+ 256*b0])
                relu(y0t[:, m, :], ps, 1.0 / WS, None if zb else b0_sb[:, m:m + 1])
            if c == 0:
                nc.sync.dma_start(w18_sb[:], w18_r)

            # ---- cross a-pass (bf16): psA rows = [v0, v1, v2, a3] ----
            psA = aps.tile([4, CHUNK], _f32, tag="a", name=f"psA_{c}")
            for k in range(KB):
                nc.tensor.matmul(
                    psA[:],
                    lhsT=cwo(k),
                    rhs=xtb_t[:, k, :],
                    start=(k == 0),
                    stop=False,
                )
            nc.tensor.matmul(
                psA[:], lhsT=vcol, rhs=ones_sb[:], start=False, stop=True
            )
            asb = asp.tile([4, CHUNK], _bf, tag="asb", name=f"asb_{c}")
            nc.scalar.activation(asb[:], psA[:], AF.Copy)
            # shuffle all four rows onto partition 0 (engines can't cross
            # partitions; the DMA crossbar can): as1[0, i, b] = a_i[b]
            as1 = asp.tile([1, 4, CHUNK], _bf, tag="as1", name=f"as1_{c}")
            nc.sync.dma_start(out=as1[:, :, :], in_=asb[:, :])
            if c == 0:
                nc.sync.dma_start(w28_sb[:], w28_r)
                nc.sync.dma_start(ow8_sb[:], ow8_d[:, :])

            # ---- deep L1 (fp8 DoubleRow); y1 in two pair-tiles so L2's
            # first DR matmul only waits on the first pair's relus ----
            y1p = [
                yp.tile([128, 2, CHUNK], _f8, tag=f"y1p{i}", name=f"y1p{i}_{c}")
                for i in range(M1 // 2)
            ]
            for m in range(M1):
                ps = dps.tile([128, CHUNK], _f32, tag="dps", name=f"ps1_{c}_{m}")
                for j in range(K8 // 2):
                    nc.tensor.matmul(
                        ps[:],
                        lhsT=w18_sb[:, 2 * j:2 * j + 2, m * 128:(m + 1) * 128],
                        rhs=y0t[:, 2 * j:2 * j + 2, :],
                        start=(j == 0),
                        stop=(j == K8 // 2 - 1),
                        perf_mode=DR,
                    )
                relu(
                    y1p[m // 2][:, m % 2, :], ps, 1.0 / WS,
                    None if zb else b1_sb[:, m:m + 1],
                )

            # ---- deep L2 (fp8 DoubleRow) -> fp8 y2 (x256 scale) ----
            y28 = yp.tile([128, M2, CHUNK], _f8, tag="y2", name=f"y2_{c}")
            for m in range(M2):
                ps = dps.tile([128, CHUNK], _f32, tag="dps", name=f"ps2_{c}_{m}")
                for j in range(M1 // 2):
                    nc.tensor.matmul(
                        ps[:],
                        lhsT=w28_sb[:, 2 * j:2 * j + 2, m * 128:(m + 1) * 128],
                        rhs=y1p[j][:, :, :],
                        start=(j == 0),
                        stop=(j == M1 // 2 - 1),
                        perf_mode=DR,
                    )
                relu(
                    y28[:, m, :], ps, 1.0 / WS,
                    None if zb else b2_sb[:, m:m + 1],
                )

            # ---- out_d: psO = 4096 * (y_deep . ow_d) [+ 4096*obP].
            # (DoubleRow with one output partition fails walrus codegen, so
            # plain fp8 matvecs.) ----
            psO = ops.tile([1, CHUNK], _f32, tag="po", name=f"psO_{c}")
            for m in range(M2):
                nc.tensor.matmul(
                    psO[:],
                    lhsT=ow8_sb[:, m, :],
                    rhs=y28[:, m, :],
                    start=(m == 0),
                    stop=(m == M2 - 1) and zo,
                )
            if not zo:
                nc.tensor.matmul(
                    psO[:], lhsT=obb, rhs=ones_sb[:], start=False, stop=True
                )

            # ---- cross combine: oc = ((v0*v1 + c1)*v2 + c2) * a3' where
            # a3' = 4096*a3 (folded on host) so oc matches psO's scale ----
            eng = nc.gpsimd
            v0 = as1[:, 0, :]
            v1 = as1[:, 1, :]
            v2 = as1[:, 2, :]
            a3 = as1[:, 3, :]
            p1 = rp.tile([1, CHUNK], _bf, tag="p1", name=f"p1_{c}")
            eng.tensor_tensor(out=p1[:], in0=v0, in1=v1, op=OP.mult)
            if not zc:
                eng.tensor_scalar_add(p1[:], p1[:], sc_sb[0:1, 0:1])
            p2 = rp.tile([1, CHUNK], _bf, tag="p2", name=f"p2_{c}")
            eng.tensor_tensor(out=p2[:], in0=p1[:], in1=v2, op=OP.mult)
            if not zc:
                eng.tensor_scalar_add(p2[:], p2[:], sc_sb[0:1, 1:2])
            oc = rp.tile([1, CHUNK], _bf, tag="oc", name=f"oc_{c}")
            eng.tensor_tensor(out=oc[:], in0=p2[:], in1=a3, op=OP.mult)
            ot = otp.tile([1, CHUNK], _f32, tag="ot", name=f"ot_{c}")
            nc.vector.tensor_tensor(out=ot[:], in0=oc[:], in1=psO[:], op=OP.add)
            nc.sync.dma_start(out=out_d[c:c + 1, :], in_=ot[:])

    nc.compile()
    return nc


def _get_nc(zb=True, zc=True, zo=True):
    key = f"nc_zb{int(zb)}_zc{int(zc)}_zo{int(zo)}"
    if key not in _CACHE:
        _CACHE[key] = _build_nc(zb=zb, zc=zc, zo=zo)
    return _CACHE[key]


def _prep_in_maps(inputs, zb, zc, zo):
    fi = np.asarray(inputs["feature_index"]).astype(np.int64)
    fvv = np.asarray(inputs["feature_value"], dtype=np.float32)
    with_fv = not bool(np.all(fvv == 1.0))
    emb = np.asarray(inputs["emb_table"], dtype=np.float32)
    cw = np.asarray(inputs["cross_w"], dtype=np.float32)
    cb = np.asarray(inputs["cross_b"], dtype=np.float32)
    w0 = np.asarray(inputs["w0"], dtype=np.float32)
    b0 = np.asarray(inputs["b0"], dtype=np.float32)
    w1 = np.asarray(inputs["w1"], dtype=np.float32)
    b1 = np.asarray(inputs["b1"], dtype=np.float32)
    w2 = np.asarray(inputs["w2"], dtype=np.float32)
    b2 = np.asarray(inputs["b2"], dtype=np.float32)
    ow = np.asarray(inputs["out_w"], dtype=np.float32).reshape(-1)
    ob = np.asarray(inputs["out_b"], dtype=np.float32).reshape(-1)

    # ---- host gather into padded, transposed layouts ----
    idxb = np.full((B, FPB), NF, dtype=np.int64)
    idxb[:, :F] = fi
    idx8 = np.full((B, FP8), NF, dtype=np.int64)
    idx8[:, :F] = fi
    if with_fv:
        embp = np.zeros((NF + 1, E), dtype=np.float32)
        embp[:NF] = emb
        xb_nat = embp[idxb]                       # [B, 28, 32] f32
        xb_nat *= np.concatenate(
            [fvv, np.ones((B, FPB - F), np.float32)], axis=1
        )[:, :, None]
        x8_nat = np.zeros((B, FP8, E), dtype=np.float32)
        x8_nat[:, :FPB] = xb_nat
        x8_nat = (x8_nat * XS).astype(_np_f8)
        xb_nat = xb_nat.astype(_np_bf)
    else:
        table_bf = np.zeros((NF + 1, E), dtype=_np_bf)
        table_bf[:NF] = emb.astype(_np_bf)
        table_f8 = np.zeros((NF + 1, E), dtype=_np_f8)
        table_f8[:NF] = (emb * XS).astype(_np_f8)
        xb_nat = table_bf[idxb]                   # [B, 28, 32] bf16
        x8_nat = table_f8[idx8]                   # [B, 32, 32] fp8

    # ---- shared (replicated) weight layouts ----
    def kpm(w, ktiles, scale):
        # [K, M] -> [128, ktiles*M] with w[p, k, m] = W[k*128+p, m]*scale
        K, M = w.shape
        wq = np.zeros((ktiles * 128, M), dtype=np.float32)
        wq[:K] = w * scale
        return np.ascontiguousarray(
            wq.reshape(ktiles, 128, M).transpose(1, 0, 2).reshape(128, ktiles * M)
        )

    w08 = kpm(w0, K8, WS).astype(_np_f8)
    w18 = kpm(w1, K8, WS).astype(_np_f8)
    w28 = kpm(w2, M1, WS).astype(_np_f8)

    wl = np.zeros((4, KB * 128), dtype=np.float32)
    wl[0, :D] = cw[0]
    wl[1, :D] = cw[1]
    wl[2, :D] = cw[2]
    wl[3, :D] = ow[:D] * 4096.0   # matches psO's 4096x scale; host divides
    cwo = wl.reshape(4, KB, 128).transpose(2, 1, 0).reshape(128, KB * 4)
    owd = ow[D:].reshape(M2, 128).T
    ow8 = np.ascontiguousarray(owd * WS).astype(_np_f8)
    C = np.cumsum(cb)
    obb = np.zeros((128, 1), dtype=np.float32)
    obb[0, 0] = (ob[0] + C[2] * ow[:D].sum()) * 4096.0
    vcol = np.zeros((128, 4), dtype=np.float32)
    vcol[0, 0:3] = 1.0
    smw = np.ascontiguousarray(
        np.concatenate([cwo, owd, obb, vcol], axis=1)
    ).astype(_np_bf)

    shared = dict(w08=w08, w18=w18, w28=w28, ow8=ow8, smw=smw)
    if not zc:
        shared["sc"] = np.array(
            [[C[0] * cw[1].sum(), C[1] * cw[2].sum()]], dtype=np.float32
        )
    if not zb:
        b0r = (b0 * XS).reshape(M0, 128).T
        b1r = (b1 * XS).reshape(M1, 128).T
        b2r = (b2 * XS).reshape(M2, 128).T
        shared["cst"] = np.ascontiguousarray(
            np.concatenate([b0r, b1r, b2r], axis=1).astype(np.float32)
        )

    in_maps = []
    for core in range(N_CORES):
        sl = slice(core * S, (core + 1) * S)
        # [S, K, 128] -> [128, K, S] -> [128, K*S]
        xtb = np.ascontiguousarray(
            xb_nat[sl].reshape(S, KB, 128).transpose(2, 1, 0).reshape(128, KB * S)
        )
        xt8 = np.ascontiguousarray(
            x8_nat[sl].reshape(S, K8, 128).transpose(2, 1, 0).reshape(128, K8 * S)
        )
        in_maps.append(dict(xtb=xtb, xt8=xt8, **shared))
    return in_maps


def _flags(inputs):
    zb = (
        bool(np.all(np.asarray(inputs["b0"]) == 0.0))
        and bool(np.all(np.asarray(inputs["b1"]) == 0.0))
        and bool(np.all(np.asarray(inputs["b2"]) == 0.0))
    )
    zc = bool(np.all(np.asarray(inputs["cross_b"]) == 0.0))
    ow = np.asarray(inputs["out_w"], dtype=np.float32).reshape(-1)
    cb = np.asarray(inputs["cross_b"], dtype=np.float32)
    obp = float(np.asarray(inputs["out_b"]).reshape(-1)[0]) + float(
        np.cumsum(cb)[2] * ow[:D].sum()
    )
    zo = obp == 0.0
    return zb, zc, zo


def _run(inputs, trace=False, **kw):
    zb, zc, zo = _flags(inputs)
    nc = _get_nc(zb=zb, zc=zc, zo=zo)
    in_maps = _prep_in_maps(inputs, zb, zc, zo)
    res = run_bass_kernel_spmd(
        nc, in_maps, core_ids=list(range(N_CORES)), trace=trace, **kw
    )
    out = np.concatenate([r["out"].reshape(S, 1) for r in res.results], axis=0)
    return out.astype(np.float32) / 4096.0, res


def kernel(**inputs) -> np.ndarray:
    out, _ = _run(inputs, trace=False)
    return out


# revision 35
# speedup vs baseline: 1.1791x; 1.1791x over previous
"""DCN (cross+deep) Trainium2 Bass kernel, 8 NeuronCores.

Sharding: data-parallel over batch (2048 rows/core); embedding gather on
host (table never touches the device); cross/deep weights replicated.

Key structure (vs the naive formulation):
  * Cross branch is algebraically collapsed: with a_i = x0 . w_i and
    a_3 = x0 . ow_cross, the full cross stack + its output contribution
    reduce to per-row scalar recurrences:
       S0 = a0; u1 = 1+S0; S1 = u1*a1 + c1; u2 = u1+S1; S2 = u2*a2 + c2;
       T = u2+S2; out_cross = T*a3 + const.
    So the PE computes ONE 7-matmul group ([128,4] lhsT) instead of
    3x7 broadcast matvecs + 7 output matvecs.
  * Deep branch runs in fp8(e4m3) with DoubleRow perf mode: each matmul
    contracts two 128-row k-tiles at 0.5 cycles/output-row. Activations
    are scaled x256 and weights x16 (exact power-of-2 descale in the
    relus), keeping everything in e4m3's normal range.
  * x ships pre-transposed from host in bf16 (cross) + fp8 (deep)
    layouts. Engine split per chunk: ACT = L0 relus + a-copy; DVE =
    L1/L2 relus + final add; Pool = cross recurrence ([4,128] layout,
    brought to partitions 0-3 by a tiny SBUF->SBUF DMA shuffle).
  * L1/L2 run k-pair-outer so they can start as soon as the first two
    producer tiles are relu'd; out_d accumulates in [4,128] PSUM groups
    so the tail is one small DVE add + DMA.
"""

import numpy as np
import ml_dtypes
from contextlib import ExitStack

import concourse.tile as tile
import concourse.mybir as mybir
from concourse import bacc
from concourse.bass_utils import run_bass_kernel_spmd

# ---- problem constants (hardcoded; kernel.py must be self-contained) ----
B, F, E = 16384, 26, 32
NF = 1_000_000
D = F * E                     # 832
DEEP = (1024, 512, 256)
N_CORES = 8
S = B // N_CORES              # 2048 rows per core
CHUNK = 512
NCHUNK = S // CHUNK           # 4
KB = 7                        # bf16 k-tiles (896 = 28 features)
K8 = 8                        # fp8 k-tiles (1024 = 32 features)
FPB, FP8 = 28, 32             # padded feature counts
M0, M1, M2 = DEEP[0] // 128, DEEP[1] // 128, DEEP[2] // 128  # 8, 4, 2
XS, WS = 256.0, 16.0          # fp8 scales for activations / weights

_bf = mybir.dt.bfloat16
_f8 = mybir.dt.float8e4
_f32 = mybir.dt.float32
_np_bf = ml_dtypes.bfloat16
_np_f8 = ml_dtypes.float8_e4m3

_CACHE = {}
DR = mybir.MatmulPerfMode.DoubleRow


def _build_nc(zb=True, zc=True, zo=True):
    """zb: deep biases all zero; zc: cross biases zero; zo: out bias zero."""
    AF = mybir.ActivationFunctionType
    OP = mybir.AluOpType
    nc = bacc.Bacc(
        "TRN2", target_bir_lowering=False, debug=False, num_devices=N_CORES
    )

    # x pre-transposed on host: xtb[p, k*S+b] = bf16(x[b, k*128+p])
    xtb_d = nc.dram_tensor("xtb", [128, KB * S], _bf, kind="ExternalInput")
    # x8[p, k*S+b] = fp8(x[b, k*128+p] * 256)
    xt8_d = nc.dram_tensor("xt8", [128, K8 * S], _f8, kind="ExternalInput")
    # deep weights fp8 (x16): w[p, k, m] = fp8(W[k*128+p, m] * 16)
    w08_d = nc.dram_tensor("w08", [128, K8 * DEEP[0]], _f8, kind="ExternalInput")
    w18_d = nc.dram_tensor("w18", [128, K8 * DEEP[1]], _f8, kind="ExternalInput")
    w28_d = nc.dram_tensor("w28", [128, M1 * DEEP[2]], _f8, kind="ExternalInput")
    # fp8 deep-out column pair (x16); psO comes out scaled by 4096 and the
    # host divides the final output by 4096 (exact power of two)
    ow8_d = nc.dram_tensor("ow8", [128, M2], _f8, kind="ExternalInput")
    # merged small weights bf16: [cwo (28) | owd (2) | obb (1) | vcol (4)]
    # vcol: partition-0 row [1,1,1,0] -- adds +1 to a0..a2 inside the a-pass
    # psum group (via a ones-rhs matmul), so psA rows become [v0,v1,v2,a3]
    # with v_i = 1 + a_i and the cross recurrence factorizes to
    #   out_cross = ((v0*v1 + c1)*v2 + c2) * a3.
    SMW = KB * 4 + M2 + 1 + 4
    smw_d = nc.dram_tensor("smw", [128, SMW], _bf, kind="ExternalInput")
    if not zc:
        sc_d = nc.dram_tensor("sc", [1, 2], _f32, kind="ExternalInput")
    if not zb:
        cst_d = nc.dram_tensor("cst", [128, M0 + M1 + M2], _f32, kind="ExternalInput")
    out_d = nc.dram_tensor("out", [NCHUNK, CHUNK], _f32, kind="ExternalOutput")

    xtb_r = xtb_d[:, :].rearrange("p (k s) -> p k s", k=KB)
    xt8_r = xt8_d[:, :].rearrange("p (k s) -> p k s", k=K8)
    w08_r = w08_d[:, :].rearrange("p (k m) -> p k m", k=K8)
    w18_r = w18_d[:, :].rearrange("p (k m) -> p k m", k=K8)
    w28_r = w28_d[:, :].rearrange("p (k m) -> p k m", k=M1)

    with ExitStack() as ctx:
        tc = ctx.enter_context(tile.TileContext(nc))
        wp = ctx.enter_context(tc.tile_pool(name="wp", bufs=1))
        xbp = ctx.enter_context(tc.tile_pool(name="xbp", bufs=2))
        x8p = ctx.enter_context(tc.tile_pool(name="x8p", bufs=2))
        yp = ctx.enter_context(tc.tile_pool(name="yp", bufs=2))
        asp = ctx.enter_context(tc.tile_pool(name="asp", bufs=2))
        rp = ctx.enter_context(tc.tile_pool(name="rp", bufs=2))
        otp = ctx.enter_context(tc.tile_pool(name="otp", bufs=2))
        dps = ctx.enter_context(tc.tile_pool(name="dps", bufs=4, space="PSUM"))
        aps = ctx.enter_context(tc.tile_pool(name="aps", bufs=1, space="PSUM"))
        ops = ctx.enter_context(tc.tile_pool(name="ops", bufs=2, space="PSUM"))

        # ---- x chunk 0 + first half of w0 first: L0 m0-3 critical path.
        # w0 lives in TWO tiles so L0 m0-3 don't wait on the second DMA
        # (tile dependency tracking is tile-granular). ----
        xt8_0 = x8p.tile([128, K8, CHUNK], _f8, tag="xt8", name="xt8_0")
        nc.sync.dma_start(xt8_0[:], xt8_r[:, :, 0:CHUNK])
        w08a_sb = wp.tile([128, K8, DEEP[0] // 2], _f8)
        w08b_sb = wp.tile([128, K8, DEEP[0] // 2], _f8)
        w18_sb = wp.tile([128, K8, DEEP[1]], _f8)
        w28_sb = wp.tile([128, M1, DEEP[2]], _f8)
        ow8_sb = wp.tile([128, M2, 1], _f8)
        nc.sync.dma_start(w08a_sb[:], w08_r[:, :, 0:512])
        nc.sync.dma_start(w08b_sb[:], w08_r[:, :, 512:1024])

        def w0l(m):  # [128, 2, 128] lhsT slice provider for L0 tile (j pair)
            t = w08a_sb if m < 4 else w08b_sb
            mm = m % 4
            return lambda j: t[:, 2 * j:2 * j + 2, mm * 128:(mm + 1) * 128]

        smw_sb = wp.tile([128, SMW], _bf)
        nc.sync.dma_start(smw_sb[:], smw_d[:, :])

        def cwo(k):  # [128, 4] lhsT for a-pass k-tile
            return smw_sb[:, k * 4:(k + 1) * 4]

        def owd(m):  # [128, 1] deep-out column
            return smw_sb[:, KB * 4 + m:KB * 4 + m + 1]

        obb = smw_sb[:, KB * 4 + M2:KB * 4 + M2 + 1]
        vcol = smw_sb[:, KB * 4 + M2 + 1:KB * 4 + M2 + 5]
        if not zc:
            sc_sb = wp.tile([1, 2], _f32)
            nc.sync.dma_start(sc_sb[:], sc_d[:, :])
        if not zb:
            cst_sb = wp.tile([128, M0 + M1 + M2], _f32)
            nc.sync.dma_start(cst_sb[:], cst_d[:, :])
            b0_sb = cst_sb[:, 0:M0]
            b1_sb = cst_sb[:, M0:M0 + M1]
            b2_sb = cst_sb[:, M0 + M1:M0 + M1 + M2]

        # ---- preamble: observe ops + PE warm-up (p-state ramp) ----
        obs = wp.tile([128, 8], _f32)
        nc.vector.tensor_copy(obs[:, 0:1], smw_sb[:, 0:1])
        nc.gpsimd.tensor_copy(obs[:, 1:2], smw_sb[:, 0:1])
        if not zc:
            nc.vector.tensor_copy(obs[:, 2:3], sc_sb[0:1, 0:1])
        nc.scalar.activation(obs[:, 3:4], smw_sb[:, 0:1], AF.Copy)
        if not zb:
            nc.scalar.activation(obs[:, 4:5], b0_sb[:, 0:1], AF.Copy)
        warm = wp.tile([128, 512], _bf)
        nc.vector.memset(warm[:], 0.0)
        ones_sb = wp.tile([128, CHUNK], _bf)
        nc.gpsimd.memset(ones_sb[:], 1.0)
        warm_ps = dps.tile([128, 512], _f32, tag="dps", name="warm_ps")
        for _ in range(8):
            nc.tensor.matmul(
                warm_ps[:], lhsT=warm[:, 0:128], rhs=warm[:], start=True, stop=True
            )
        # NOTE: only touch tensors whose DMAs are emitted BEFORE this point —
        # touching late-loaded weights stalls the in-order PE stream on their
        # DMA semaphores.
        dummy_ps = ops.tile([1, 8], _f32, tag="dummy", bufs=1)
        touch = [
            w08a_sb[:, 0, 0:1],
            smw_sb[:, 0:1],
            ones_sb[:, 0:1],
        ]
        for w_ap in touch:
            nc.tensor.matmul(dummy_ps[0:1, 0:1], lhsT=w_ap, rhs=w_ap, start=True, stop=True)

        HH = CHUNK // 2

        def relu(out_ap, ps, scale, bias_col):
            # fp8/bf16 relu of a [128, CHUNK] psum tile, split column-wise
            # across ACT and DVE so output latency ~ half an op.
            if zb:
                nc.scalar.activation(
                    out_ap[:, 0:HH], ps[:, 0:HH], AF.Relu, scale=scale
                )
                nc.vector.tensor_scalar(
                    out_ap[:, HH:], ps[:, HH:], scale, 0.0, OP.mult, OP.max
                )
            else:
                nc.scalar.activation(
                    out_ap[:, :], ps[:, :], AF.Relu, bias=bias_col, scale=scale
                )

        for c in range(NCHUNK):
            cs = c * CHUNK
            if c == 0:
                xt8_t = xt8_0
            else:
                xt8_t = x8p.tile([128, K8, CHUNK], _f8, tag="xt8", name=f"xt8_{c}")
                nc.sync.dma_start(xt8_t[:], xt8_r[:, :, cs:cs + CHUNK])
            xtb_t = xbp.tile([128, KB, CHUNK], _bf, tag="xtb", name=f"xtb_{c}")
            nc.sync.dma_start(xtb_t[:], xtb_r[:, :, cs:cs + CHUNK])

            # ---- deep L0 (fp8 DoubleRow), psum = h0 * 4096 ----
            y0t = yp.tile([128, K8, CHUNK], _f8, tag="y0", name=f"y0_{c}")
            for m in range(M0):
                ps = dps.tile([128, CHUNK], _f32, tag="dps", name=f"ps0_{c}_{m}")
                lhs = w0l(m)
                for j in range(K8 // 2):
                    nc.tensor.matmul(
                        ps[:],
                        lhsT=lhs(j),
                        rhs=xt8_t[:, 2 * j:2 * j + 2, :],
                        start=(j == 0),
                        stop=(j == K8 // 2 - 1),
                        perf_mode=DR,
                    )
                # y0 = fp8(relu(h0)*256) = relu(psum/16 [+ 256*b0])
                relu(y0t[:, m, :], ps, 1.0 / WS, None if zb else b0_sb[:, m:m + 1])
            if c == 0:
                nc.sync.dma_start(w18_sb[:], w18_r)

            # ---- cross a-pass (bf16): psA rows = [v0, v1, v2, a3] ----
            psA = aps.tile([4, CHUNK], _f32, tag="a", name=f"psA_{c}")
            for k in range(KB):
                nc.tensor.matmul(
                    psA[:],
                    lhsT=cwo(k),
                    rhs=xtb_t[:, k, :],
                    start=(k == 0),
                    stop=False,
                )
            nc.tensor.matmul(
                psA[:], lhsT=vcol, rhs=ones_sb[:], start=False, stop=True
            )
            asb = asp.tile([4, CHUNK], _bf, tag="asb", name=f"asb_{c}")
            nc.scalar.activation(asb[:], psA[:], AF.Copy)
            # shuffle all four rows onto partition 0 (engines can't cross
            # partitions; the DMA crossbar can): as1[0, i, b] = a_i[b]
            as1 = asp.tile([1, 4, CHUNK], _bf, tag="as1", name=f"as1_{c}")
            nc.sync.dma_start(out=as1[:, :, :], in_=asb[:, :])
            if c == 0:
                nc.sync.dma_start(w28_sb[:], w28_r)
                nc.sync.dma_start(ow8_sb[:], ow8_d[:, :])

            # ---- deep L1 (fp8 DoubleRow); y1 in two pair-tiles so L2's
            # first DR matmul only waits on the first pair's relus ----
            y1p = [
                yp.tile([128, 2, CHUNK], _f8, tag=f"y1p{i}", name=f"y1p{i}_{c}")
                for i in range(M1 // 2)
            ]
            for m in range(M1):
                ps = dps.tile([128, CHUNK], _f32, tag="dps", name=f"ps1_{c}_{m}")
                for j in range(K8 // 2):
                    nc.tensor.matmul(
                        ps[:],
                        lhsT=w18_sb[:, 2 * j:2 * j + 2, m * 128:(m + 1) * 128],
                        rhs=y0t[:, 2 * j:2 * j + 2, :],
                        start=(j == 0),
                        stop=(j == K8 // 2 - 1),
                        perf_mode=DR,
                    )
                relu(
                    y1p[m // 2][:, m % 2, :], ps, 1.0 / WS,
                    None if zb else b1_sb[:, m:m + 1],
                )

            # ---- deep L2 (fp8 DoubleRow) -> fp8 y2 (x256 scale) ----
            y28 = yp.tile([128, M2, CHUNK], _f8, tag="y2", name=f"y2_{c}")
            for m in range(M2):
                ps = dps.tile([128, CHUNK], _f32, tag="dps", name=f"ps2_{c}_{m}")
                for j in range(M1 // 2):
                    nc.tensor.matmul(
                        ps[:],
                        lhsT=w28_sb[:, 2 * j:2 * j + 2, m * 128:(m + 1) * 128],
                        rhs=y1p[j][:, :, :],
                        start=(j == 0),
                        stop=(j == M1 // 2 - 1),
                        perf_mode=DR,
                    )
                relu(
                    y28[:, m, :], ps, 1.0 / WS,
                    None if zb else b2_sb[:, m:m + 1],
                )

            # ---- out_d: psO = 4096 * (y_deep . ow_d) [+ 4096*obP].
            # (DoubleRow with one output partition fails walrus codegen, so
            # plain fp8 matvecs.) ----
            psO = ops.tile([1, CHUNK], _f32, tag="po", name=f"psO_{c}")
            for m in range(M2):
                nc.tensor.matmul(
                    psO[:],
                    lhsT=ow8_sb[:, m, :],
                    rhs=y28[:, m, :],
                    start=(m == 0),
                    stop=(m == M2 - 1) and zo,
                )
            if not zo:
                nc.tensor.matmul(
                    psO[:], lhsT=obb, rhs=ones_sb[:], start=False, stop=True
                )

            # ---- cross combine: oc = ((v0*v1 + c1)*v2 + c2) * a3' where
            # a3' = 4096*a3 (folded on host) so oc matches psO's scale ----
            eng = nc.gpsimd
            v0 = as1[:, 0, :]
            v1 = as1[:, 1, :]
            v2 = as1[:, 2, :]
            a3 = as1[:, 3, :]
            p1 = rp.tile([1, CHUNK], _bf, tag="p1", name=f"p1_{c}")
            eng.tensor_tensor(out=p1[:], in0=v0, in1=v1, op=OP.mult)
            if not zc:
                eng.tensor_scalar_add(p1[:], p1[:], sc_sb[0:1, 0:1])
            p2 = rp.tile([1, CHUNK], _bf, tag="p2", name=f"p2_{c}")
            eng.tensor_tensor(out=p2[:], in0=p1[:], in1=v2, op=OP.mult)
            if not zc:
                eng.tensor_scalar_add(p2[:], p2[:], sc_sb[0:1, 1:2])
            oc = rp.tile([1, CHUNK], _bf, tag="oc", name=f"oc_{c}")
            eng.tensor_tensor(out=oc[:], in0=p2[:], in1=a3, op=OP.mult)
            ot = otp.tile([1, CHUNK], _f32, tag="ot", name=f"ot_{c}")
            nc.vector.tensor_tensor(out=ot[:], in0=oc[:], in1=psO[:], op=OP.add)
            nc.sync.dma_start(out=out_d[c:c + 1, :], in_=ot[:])

    nc.compile()
    return nc


def _get_nc(zb=True, zc=True, zo=True):
    key = f"nc_zb{int(zb)}_zc{int(zc)}_zo{int(zo)}"
    if key not in _CACHE:
        _CACHE[key] = _build_nc(zb=zb, zc=zc, zo=zo)
    return _CACHE[key]


def _prep_in_maps(inputs, zb, zc, zo):
    fi = np.asarray(inputs["feature_index"]).astype(np.int64)
    fvv = np.asarray(inputs["feature_value"], dtype=np.float32)
    with_fv = not bool(np.all(fvv == 1.0))
    emb = np.asarray(inputs["emb_table"], dtype=np.float32)
    cw = np.asarray(inputs["cross_w"], dtype=np.float32)
    cb = np.asarray(inputs["cross_b"], dtype=np.float32)
    w0 = np.asarray(inputs["w0"], dtype=np.float32)
    b0 = np.asarray(inputs["b0"], dtype=np.float32)
    w1 = np.asarray(inputs["w1"], dtype=np.float32)
    b1 = np.asarray(inputs["b1"], dtype=np.float32)
    w2 = np.asarray(inputs["w2"], dtype=np.float32)
    b2 = np.asarray(inputs["b2"], dtype=np.float32)
    ow = np.asarray(inputs["out_w"], dtype=np.float32).reshape(-1)
    ob = np.asarray(inputs["out_b"], dtype=np.float32).reshape(-1)

    # ---- host gather into padded, transposed layouts ----
    idxb = np.full((B, FPB), NF, dtype=np.int64)
    idxb[:, :F] = fi
    idx8 = np.full((B, FP8), NF, dtype=np.int64)
    idx8[:, :F] = fi
    if with_fv:
        embp = np.zeros((NF + 1, E), dtype=np.float32)
        embp[:NF] = emb
        xb_nat = embp[idxb]                       # [B, 28, 32] f32
        xb_nat *= np.concatenate(
            [fvv, np.ones((B, FPB - F), np.float32)], axis=1
        )[:, :, None]
        x8_nat = np.zeros((B, FP8, E), dtype=np.float32)
        x8_nat[:, :FPB] = xb_nat
        x8_nat = (x8_nat * XS).astype(_np_f8)
        xb_nat = xb_nat.astype(_np_bf)
    else:
        table_bf = np.zeros((NF + 1, E), dtype=_np_bf)
        table_bf[:NF] = emb.astype(_np_bf)
        table_f8 = np.zeros((NF + 1, E), dtype=_np_f8)
        table_f8[:NF] = (emb * XS).astype(_np_f8)
        xb_nat = table_bf[idxb]                   # [B, 28, 32] bf16
        x8_nat = table_f8[idx8]                   # [B, 32, 32] fp8

    # ---- shared (replicated) weight layouts ----
    def kpm(w, ktiles, scale):
        # [K, M] -> [128, ktiles*M] with w[p, k, m] = W[k*128+p, m]*scale
        K, M = w.shape
        wq = np.zeros((ktiles * 128, M), dtype=np.float32)
        wq[:K] = w * scale
        return np.ascontiguousarray(
            wq.reshape(ktiles, 128, M).transpose(1, 0, 2).reshape(128, ktiles * M)
        )

    w08 = kpm(w0, K8, WS).astype(_np_f8)
    w18 = kpm(w1, K8, WS).astype(_np_f8)
    w28 = kpm(w2, M1, WS).astype(_np_f8)

    wl = np.zeros((4, KB * 128), dtype=np.float32)
    wl[0, :D] = cw[0]
    wl[1, :D] = cw[1]
    wl[2, :D] = cw[2]
    wl[3, :D] = ow[:D] * 4096.0   # matches psO's 4096x scale; host divides
    cwo = wl.reshape(4, KB, 128).transpose(2, 1, 0).reshape(128, KB * 4)
    owd = ow[D:].reshape(M2, 128).T
    ow8 = np.ascontiguousarray(owd * WS).astype(_np_f8)
    C = np.cumsum(cb)
    obb = np.zeros((128, 1), dtype=np.float32)
    obb[0, 0] = (ob[0] + C[2] * ow[:D].sum()) * 4096.0
    vcol = np.zeros((128, 4), dtype=np.float32)
    vcol[0, 0:3] = 1.0
    smw = np.ascontiguousarray(
        np.concatenate([cwo, owd, obb, vcol], axis=1)
    ).astype(_np_bf)

    shared = dict(w08=w08, w18=w18, w28=w28, ow8=ow8, smw=smw)
    if not zc:
        shared["sc"] = np.array(
            [[C[0] * cw[1].sum(), C[1] * cw[2].sum()]], dtype=np.float32
        )
    if not zb:
        b0r = (b0 * XS).reshape(M0, 128).T
        b1r = (b1 * XS).reshape(M1, 128).T
        b2r = (b2 * XS).reshape(M2, 128).T
        shared["cst"] = np.ascontiguousarray(
            np.concatenate([b0r, b1r, b2r], axis=1).astype(np.float32)
        )

    in_maps = []
    for core in range(N_CORES):
        sl = slice(core * S, (core + 1) * S)
        # [S, K, 128] -> [128, K, S] -> [128, K*S]
        xtb = np.ascontiguousarray(
            xb_nat[sl].reshape(S, KB, 128).transpose(2, 1, 0).reshape(128, KB * S)
        )
        xt8 = np.ascontiguousarray(
            x8_nat[sl].reshape(S, K8, 128).transpose(2, 1, 0).reshape(128, K8 * S)
        )
        in_maps.append(dict(xtb=xtb, xt8=xt8, **shared))
    return in_maps


def _flags(inputs):
    zb = (
        bool(np.all(np.asarray(inputs["b0"]) == 0.0))
        and bool(np.all(np.asarray(inputs["b1"]) == 0.0))
        and bool(np.all(np.asarray(inputs["b2"]) == 0.0))
    )
    zc = bool(np.all(np.asarray(inputs["cross_b"]) == 0.0))
    ow = np.asarray(inputs["out_w"], dtype=np.float32).reshape(-1)
    cb = np.asarray(inputs["cross_b"], dtype=np.float32)
    obp = float(np.asarray(inputs["out_b"]).reshape(-1)[0]) + float(
        np.cumsum(cb)[2] * ow[:D].sum()
    )
    zo = obp == 0.0
    return zb, zc, zo


def _run(inputs, trace=False, **kw):
    zb, zc, zo = _flags(inputs)
    nc = _get_nc(zb=zb, zc=zc, zo=zo)
    in_maps = _prep_in_maps(inputs, zb, zc, zo)
    res = run_bass_kernel_spmd(
        nc, in_maps, core_ids=list(range(N_CORES)), trace=trace, **kw
    )
    out = np.concatenate([r["out"].reshape(S, 1) for r in res.results], axis=0)
    return out.astype(np.float32) / 4096.0, res


def kernel(**inputs) -> np.ndarray:
    out, _ = _run(inputs, trace=False)
    return out
